# revision 1
# baseline (speedup 1.0000x reference)
"""GQA attention kernel for Trainium2, 8 NeuronCores.

Problem: B=2, T=2048, D=1024, 16 Q heads / 4 KV heads, head_dim=64, RoPE,
causal softmax, out-projection.

Sharding: 8 cores = 2 (batch) x 4 (KV group). Core c handles batch c//4 and
KV group g=c%4 (query heads 4g..4g+3). wq/wk/wv column-sharded, wo
row-sharded; the 4 partial outputs per batch are summed on the host.

On-chip layout: everything is kept transposed (head_dim on partitions):
  xT (D, T), qT (256, T), kT (64, T).  Scores are computed directly in
transposed orientation scoresT[j, i] = k_j . q_i (j on partitions), so no
on-chip transposes of the attention matrix are needed.  Softmax runs without
max-subtraction (scores are O(6) bounded), and the denominator L[i] is
obtained for free by augmenting V with a ones-column in the PV matmul.
RoPE pairs are de-interleaved via a host-side column permutation of wq/wk so
rotate-half applies; the interleave never needs to be undone because q and k
share the same permutation and V/out stay in natural order.

All matmuls run as float32r (full fp32 data, fast PE mode).  Engines have no
cross-partition paths, so every partition-base change (rotate-half swap, kT
duplication, odd-head placement) goes through SBUF->SBUF DMA.
"""

import numpy as np
import sys

sys.path.insert(0, "/opt/trn_rl_repo")

from concourse import bass, bacc, mybir, tile  # noqa: E402
from concourse.bass_utils import run_bass_kernel_spmd  # noqa: E402

F32 = mybir.dt.float32
F32R = mybir.dt.float32r

B, T, D = 2, 2048, 1024
HD = 64                      # head dim
NQH = 4                      # query heads per core
QCOLS = NQH * HD             # 256
KC = D // 128                # 8 contraction chunks
NT = T // 128                # 16 row tiles
NC4 = T // 512               # 4 512-wide column chunks
N_CORES = 8

_cache = {}


def _r(ap):
    return ap.bitcast(F32R)


def build_nc():
    """Build the (SPMD-identical) single-core bass program."""
    nc = bacc.Bacc("TRN2", target_bir_lowering=False, debug=False)

    xT_d = nc.declare_dram_parameter("xT", [D, T], F32R, isOutput=False)
    wq_d = nc.declare_dram_parameter("wq", [D, QCOLS], F32R, isOutput=False)
    wk_d = nc.declare_dram_parameter("wk", [D, HD], F32R, isOutput=False)
    wv_d = nc.declare_dram_parameter("wv", [D, HD], F32R, isOutput=False)
    wo_d = nc.declare_dram_parameter("wo", [QCOLS, D], F32R, isOutput=False)
    cos_d = nc.declare_dram_parameter("cosf", [128, T], F32, isOutput=False)
    sin_d = nc.declare_dram_parameter("sinf", [128, T], F32, isOutput=False)
    msk_d = nc.declare_dram_parameter("msk", [128, 4, 512], F32, isOutput=False)
    one_d = nc.declare_dram_parameter("onec", [128, HD], F32R, isOutput=False)
    out_d = nc.declare_dram_parameter("out", [T, D], F32, isOutput=True)

    with tile.TileContext(nc) as tc:
        with tc.tile_pool(name="sb", bufs=1) as sb:
            wq = sb.tile([128, KC, QCOLS], F32, tag="wq")
            wk = sb.tile([128, KC, HD], F32, tag="wk")
            wv = sb.tile([128, KC, HD], F32, tag="wv")
            wo = sb.tile([128, 2, D], F32, tag="wo")
            cosf = sb.tile([128, T], F32, tag="cosf")
            sinf = sb.tile([128, T], F32, tag="sinf")
            msk = sb.tile([128, 4, 512], F32, tag="msk")
            # ones row placed at partition 64 to align with the L row of the
            # PV accumulator (engines need matching partition bases).
            ones = sb.tile([65, HD], F32, tag="ones")
            qT = [sb.tile([128, T], F32, tag=f"qT{hp}", name=f"qT{hp}")
                  for hp in range(2)]
            # kT duplicated into both partition halves so scores matmuls can
            # read it at base partition 0 (even heads) or 64 (odd heads).
            kT = sb.tile([128, T], F32, tag="kT")
            v = sb.tile([128, NT, HD + 1], F32, tag="v")
            ao = [sb.tile([128, T], F32, tag=f"ao{hp}", name=f"ao{hp}")
                  for hp in range(2)]

            for k in range(KC):
                nc.sync.dma_start(_r(wq[:, k, :]), wq_d[k * 128:(k + 1) * 128, :])
                nc.sync.dma_start(_r(wk[:, k, :]), wk_d[k * 128:(k + 1) * 128, :])
                nc.sync.dma_start(_r(wv[:, k, :]), wv_d[k * 128:(k + 1) * 128, :])
            nc.sync.dma_start(cosf[:], cos_d[:])
            nc.sync.dma_start(sinf[:], sin_d[:])
            nc.sync.dma_start(msk[:], msk_d[:])
            for c in range(2):
                nc.sync.dma_start(_r(wo[:, c, :]), wo_d[c * 128:(c + 1) * 128, :])

            nc.sync.dma_start(_r(ones[64:65, :]), one_d[64:65, :])
            nc.sync.dma_start(_r(v[:, :, HD:HD + 1]), one_d[:, 0:NT])

            # --- projections (xT lives only here) ---
            with (
                tc.tile_pool(name="sbx", bufs=1) as sbx,
                tc.tile_pool(name="rope", bufs=1) as rope_pool,
                tc.tile_pool(name="ppsum", bufs=2, space="PSUM") as ppsum,
            ):
                xT = sbx.tile([128, KC, T], F32, tag="xT")
                for k in range(KC):
                    nc.sync.dma_start(_r(xT[:, k, :]), xT_d[k * 128:(k + 1) * 128, :])

                def rope_inplace(q_ap, nrows):
                    """q = q*cos + rot_half(q)*sin, on de-interleaved rows."""
                    rot = rope_pool.tile([128, T], F32, tag="rot")
                    for blk in range(nrows // 64):
                        r0 = blk * 64
                        nc.sync.dma_start(rot[r0:r0 + 32, :],
                                          q_ap[r0 + 32:r0 + 64, :])
                        nc.sync.dma_start(rot[r0 + 32:r0 + 64, :],
                                          q_ap[r0:r0 + 32, :])
                    nc.vector.tensor_mul(_r(q_ap[0:nrows, :]), q_ap[0:nrows, :],
                                         cosf[0:nrows, :])
                    nc.vector.tensor_mul(rot[0:nrows, :], rot[0:nrows, :],
                                         sinf[0:nrows, :])
                    nc.vector.tensor_add(_r(q_ap[0:nrows, :]), q_ap[0:nrows, :],
                                         rot[0:nrows, :])

                for hp in range(2):
                    pq = ppsum.tile([128, T], F32, tag="proj")
                    for ci in range(NC4):
                        cs = slice(ci * 512, (ci + 1) * 512)
                        for k in range(KC):
                            nc.tensor.matmul(
                                pq[:, cs],
                                _r(wq[:, k, hp * 128:(hp + 1) * 128]),
                                _r(xT[:, k, cs]),
                                start=(k == 0), stop=(k == KC - 1))
                    nc.scalar.copy(_r(qT[hp][:]), pq[:])
                    rope_inplace(qT[hp][:], 128)

                pk = ppsum.tile([64, T], F32, tag="proj")
                for ci in range(NC4):
                    cs = slice(ci * 512, (ci + 1) * 512)
                    for k in range(KC):
                        nc.tensor.matmul(
                            pk[:, cs], _r(wk[:, k, :]), _r(xT[:, k, cs]),
                            start=(k == 0), stop=(k == KC - 1))
                nc.scalar.copy(_r(kT[0:64, :]), pk[:])
                rope_inplace(kT[:], 64)
                nc.sync.dma_start(_r(kT[64:128, :]), _r(kT[0:64, :]))

                for t in range(NT):
                    pv = ppsum.tile([128, HD], F32, tag="proj")
                    for k in range(KC):
                        nc.tensor.matmul(
                            pv[:], _r(xT[:, k, t * 128:(t + 1) * 128]),
                            _r(wv[:, k, :]),
                            start=(k == 0), stop=(k == KC - 1))
                    nc.scalar.copy(_r(v[:, t, 0:HD]), pv[:])

            # --- attention, one KV head (4 query heads) ---
            with (
                tc.tile_pool(name="aox", bufs=2) as aox,
                tc.tile_pool(name="at", bufs=6) as at_pool,
                tc.tile_pool(name="pvpsum", bufs=1, space="PSUM") as pvp,
                tc.tile_pool(name="scpsum", bufs=3, space="PSUM") as scp,
            ):
                for h in range(NQH):
                    hp, hr = divmod(h, 2)
                    qrow = slice(hr * 64, hr * 64 + 64)
                    pv_acc = pvp.tile([HD + 1, T], F32, tag="pv")
                    for ci in range(NC4):
                        cs = slice(ci * 512, (ci + 1) * 512)
                        n_tj = (ci + 1) * 4
                        for tj in range(n_tj):
                            sc = scp.tile([128, 512], F32, tag="sc")
                            nc.tensor.matmul(
                                sc[:],
                                _r(kT[qrow, tj * 128:(tj + 1) * 128]),
                                _r(qT[hp][qrow, cs]),
                                start=True, stop=True)
                            if tj >= ci * 4:  # diagonal block: causal mask
                                nc.vector.tensor_add(
                                    sc[:], sc[:], msk[:, tj - ci * 4, :])
                            at = at_pool.tile([128, 512], F32, tag="at")
                            nc.scalar.activation(
                                _r(at[:]), sc[:],
                                mybir.ActivationFunctionType.Exp,
                                scale=0.125)
                            nc.tensor.matmul(
                                pv_acc[:, cs], _r(v[:, tj, :]), _r(at[:]),
                                start=(tj == 0), stop=(tj == n_tj - 1))
                    # normalize: ao rows of head h = pv_acc[0:64] * (1/L);
                    # L sits in pv_acc row 64 (the ones-column of v_aug).
                    linv = aox.tile([65, T], F32, tag="linv")
                    with nc.allow_low_precision(reason="fp32r linv"):
                        nc.vector.reciprocal(_r(linv[64:65, :]),
                                             pv_acc[HD:HD + 1, :])
                    if hr == 0:
                        dst = ao[hp][0:64, :]
                    else:
                        dst = aox.tile([64, T], F32, tag="aotmp")
                    nc.scalar.copy(_r(dst), pv_acc[0:HD, :])
                    for ci in range(NC4):
                        cs = slice(ci * 512, (ci + 1) * 512)
                        lb = scp.tile([HD, 512], F32, tag="sc")
                        nc.tensor.matmul(lb[:], _r(ones[64:65, :]),
                                         _r(linv[64:65, cs]),
                                         start=True, stop=True)
                        nc.vector.tensor_mul(_r(dst[:, cs]), dst[:, cs], lb[:])
                    if hr == 1:
                        nc.sync.dma_start(_r(ao[hp][64:128, :]), _r(dst))

            # --- output projection ---
            with (
                tc.tile_pool(name="outp", bufs=3) as outp,
                tc.tile_pool(name="wopsum", bufs=2, space="PSUM") as wop,
            ):
                for t in range(NT):
                    po = wop.tile([128, D], F32, tag="po")
                    for nh in range(2):
                        ns = slice(nh * 512, (nh + 1) * 512)
                        for cc in range(2):
                            nc.tensor.matmul(
                                po[:, ns],
                                _r(ao[cc][:, t * 128:(t + 1) * 128]),
                                _r(wo[:, cc, ns]),
                                start=(cc == 0), stop=(cc == 1))
                    ot = outp.tile([128, D], F32, tag="ot")
                    nc.scalar.copy(ot[:], po[:])
                    nc.sync.dma_start(out_d[t * 128:(t + 1) * 128, :], ot[:])

    nc.compile()
    return nc


def _round_f32r(a):
    """Round fp32 to the fp32r grid (11-bit mantissa, round-to-nearest)."""
    bits = np.ascontiguousarray(a, np.float32).view(np.uint32)
    return ((bits + 0x800) & 0xFFFFF000).view(np.float32)


def make_in_maps(x, freqs_cos, freqs_sin, wq, wk, wv, wo):
    """Host-side sharding + layout prep. Returns per-core input dicts."""
    x = np.asarray(x, np.float32)
    fc = np.asarray(freqs_cos, np.float32)
    fs = np.asarray(freqs_sin, np.float32)
    wq = np.asarray(wq, np.float32)
    wk = np.asarray(wk, np.float32)
    wv = np.asarray(wv, np.float32)
    wo = np.asarray(wo, np.float32)

    perm = np.concatenate([np.arange(0, HD, 2), np.arange(1, HD, 2)])
    cosT = np.ascontiguousarray(fc.T)            # (32, T)
    sinT = np.ascontiguousarray(fs.T)
    cosf = np.concatenate([cosT] * 4, axis=0)    # (128, T)
    sinf = np.concatenate([-sinT, sinT, -sinT, sinT], axis=0)

    jj = np.arange(128)[:, None]
    ii = np.arange(512)[None, :]
    msk = np.stack(
        [np.where(r * 128 + jj <= ii, 0.0, -1e30) for r in range(4)], axis=0
    ).astype(np.float32)                         # (4, 128, 512)
    mskT = np.ascontiguousarray(msk.transpose(1, 0, 2))  # (128, 4, 512)

    in_maps = []
    for c in range(N_CORES):
        b, g = divmod(c, 4)
        wq_c = wq[:, g * QCOLS:(g + 1) * QCOLS]
        wq_c = np.ascontiguousarray(
            wq_c.reshape(D, NQH, HD)[:, :, perm].reshape(D, QCOLS))
        wk_c = np.ascontiguousarray(wk[:, g * HD:(g + 1) * HD][:, perm])
        wv_c = np.ascontiguousarray(wv[:, g * HD:(g + 1) * HD])
        wo_c = np.ascontiguousarray(wo[g * QCOLS:(g + 1) * QCOLS, :])
        xT_c = np.ascontiguousarray(x[b].T)
        in_maps.append({
            "xT": _round_f32r(xT_c), "wq": _round_f32r(wq_c),
            "wk": _round_f32r(wk_c), "wv": _round_f32r(wv_c),
            "wo": _round_f32r(wo_c),
            "cosf": cosf, "sinf": sinf, "msk": mskT,
            "onec": np.ones((128, HD), np.float32),
        })
    return in_maps


def run_on_cores(in_maps, trace=False, **kwargs):
    if "nc" not in _cache:
        _cache["nc"] = build_nc()
    return run_bass_kernel_spmd(
        _cache["nc"], in_maps, core_ids=list(range(N_CORES)), trace=trace,
        **kwargs)


def kernel(x, freqs_cos, freqs_sin, wq, wk, wv, wo):
    in_maps = make_in_maps(x, freqs_cos, freqs_sin, wq, wk, wv, wo)
    res = run_on_cores(in_maps)
    outs = [res.results[c]["out"] for c in range(N_CORES)]
    full = np.empty((B, T, D), np.float32)
    for b in range(B):
        full[b] = outs[4 * b] + outs[4 * b + 1] + outs[4 * b + 2] + outs[4 * b + 3]
    return full



# revision 6
# speedup vs baseline: 1.1629x; 1.1629x over previous
"""GQA attention kernel for Trainium2, 8 NeuronCores — fp16 version.

Problem: B=2, T=2048, D=1024, 16 Q heads / 4 KV heads, head_dim=64, RoPE,
causal softmax, out-projection.

Sharding: 8 cores = 2 (batch) x 4 (KV group). Core c handles batch c//4 and
KV group g=c%4 (query heads 4g..4g+3). wq/wk/wv column-sharded, wo
row-sharded; the 4 partial outputs per batch are summed on the host.

All matmul operands are fp16 (PSUM accumulation fp32); validated numerics:
max rel err ~6e-4 vs fp64 reference. Every stationary operand is padded to
128 columns so the compiler's Fast Weight Load kicks in (fp16 + 128 cols)
and LDWEIGHTS overlaps the previous matmul.

Layout is transposed throughout (head_dim on partitions): xT (D,T),
qT (256,T), kT (64,T dup to 128), scoresT[j,i] = k_j.q_i. Softmax uses
exp(s/8 - 4): the -4 bias guards fp16 range of the unnormalized weights and
cancels exactly through the 1/L normalization. The causal mask is a 0/1
fp16 multiply on the diagonal blocks AFTER exp (cheaper than the fp32
-inf add before it). L rides the PV matmul as a ones-column of v; its
reciprocal is computed lane-parallel by packing the L row [1,512] into
[128,4] via SBUF DMA. exp runs as one ACTIVATE per two score blocks
(PSUM-contiguous) to amortize the 352-cycle ACT fixed cost.

Loop order is ci-outer (512 query columns) / head-inner, with the output
projection of each ci's row tiles interleaved right after, so the PE never
drains during ACT-heavy stretches (keeps the HAM clock gate at 2.4 GHz).
"""

import numpy as np
import sys

sys.path.insert(0, "/opt/trn_rl_repo")

from concourse import bass, bacc, mybir, tile  # noqa: E402
from concourse.bass_utils import run_bass_kernel_spmd  # noqa: E402

F32 = mybir.dt.float32
F16 = mybir.dt.float16

B, T, D = 2, 2048, 1024
HD = 64                      # head dim
NQH = 4                      # query heads per core
QCOLS = NQH * HD             # 256
KC = D // 128                # 8 contraction chunks
NT = T // 128                # 16 row tiles
NC4 = T // 512               # 4 512-wide column chunks
N_CORES = 8
G = 2                        # score blocks per exp group

_cache = {}


def build_nc():
    """Build the (SPMD-identical) single-core bass program."""
    nc = bacc.Bacc("TRN2", target_bir_lowering=False, debug=False)

    xT_d = nc.declare_dram_parameter("xT", [D, T], F16, isOutput=False)
    wq_d = nc.declare_dram_parameter("wq", [D, QCOLS], F16, isOutput=False)
    wk_d = nc.declare_dram_parameter("wk", [D, 128], F16, isOutput=False)
    wv_d = nc.declare_dram_parameter("wv", [D, 128], F16, isOutput=False)
    wo_d = nc.declare_dram_parameter("wo", [QCOLS, D], F16, isOutput=False)
    cos_d = nc.declare_dram_parameter("cosf", [128, T], F16, isOutput=False)
    sin_d = nc.declare_dram_parameter("sinf", [128, T], F16, isOutput=False)
    msk_d = nc.declare_dram_parameter("msk", [128, 4, 512], F16, isOutput=False)
    idn_d = nc.declare_dram_parameter("iden", [64, 64], F16, isOutput=False)
    out_d = nc.declare_dram_parameter("out", [T, D], F16, isOutput=True)

    with tile.TileContext(nc) as tc:
        with tc.tile_pool(name="sb", bufs=1) as sb:
            wq = sb.tile([128, KC, QCOLS], F16, tag="wq")
            wk = sb.tile([128, KC, 128], F16, tag="wk")
            wv = sb.tile([128, KC, 128], F16, tag="wv")
            wo = sb.tile([128, 2, D], F16, tag="wo")
            cosf = sb.tile([128, T], F16, tag="cosf")
            sinf = sb.tile([128, T], F16, tag="sinf")
            msk = sb.tile([128, 4, 512], F16, tag="msk")
            iden = sb.tile([64, 64], F16, tag="iden")
            onesr = sb.tile([1, 128], F16, tag="onesr")
            bias = sb.tile([128, 1], F32, tag="bias")
            qT = [sb.tile([128, T], F16, tag=f"qT{hp}", name=f"qT{hp}")
                  for hp in range(2)]
            # kT duplicated into both partition halves so scores matmuls can
            # read it at base partition 0 (even heads) or 64 (odd heads).
            kT = sb.tile([128, T], F16, tag="kT")
            vT = sb.tile([64, T], F16, tag="vT")
            # v: keys on partitions; col 64 = ones (L row), 65..127 zero pad
            # (stationary padded to 128 cols for FWL).
            v = sb.tile([128, NT, 128], F16, tag="v")
            # per-(hp, ci) attention-out tiles, rows = 2 heads x 64 dims
            ao = [[sb.tile([128, 512], F16, tag=f"ao{hp}_{ci}",
                           name=f"ao{hp}_{ci}") for ci in range(NC4)]
                  for hp in range(2)]

            nc.gpsimd.memset(onesr[:], 1.0)
            nc.gpsimd.memset(bias[:], -4.0)
            nc.gpsimd.memset(v[:], 0.0)
            nc.gpsimd.memset(v[:, :, HD:HD + 1], 1.0)

            for k in range(KC):
                nc.sync.dma_start(wq[:, k, :], wq_d[k * 128:(k + 1) * 128, :])
                nc.sync.dma_start(wk[:, k, :], wk_d[k * 128:(k + 1) * 128, :])
                nc.sync.dma_start(wv[:, k, :], wv_d[k * 128:(k + 1) * 128, :])
            nc.sync.dma_start(cosf[:], cos_d[:])
            nc.sync.dma_start(sinf[:], sin_d[:])
            nc.sync.dma_start(msk[:], msk_d[:])
            nc.sync.dma_start(iden[:], idn_d[:])
            for c in range(2):
                nc.sync.dma_start(wo[:, c, :], wo_d[c * 128:(c + 1) * 128, :])

            # --- projections (xT lives only here) ---
            with (
                tc.tile_pool(name="sbx", bufs=1) as sbx,
                tc.tile_pool(name="rope", bufs=1) as rope_pool,
                tc.tile_pool(name="ppsum", bufs=3, space="PSUM") as ppsum,
            ):
                xT = sbx.tile([128, KC, T], F16, tag="xT")
                for k in range(KC):
                    nc.sync.dma_start(xT[:, k, :], xT_d[k * 128:(k + 1) * 128, :])

                # K projection -> kT rows 0:64 (wk padded, rows 64:127 zero)
                for ci in range(NC4):
                    cs = slice(ci * 512, (ci + 1) * 512)
                    pk = ppsum.tile([128, 512], F32, tag="proj")
                    for k in range(KC):
                        nc.tensor.matmul(pk[:], wk[:, k, :], xT[:, k, cs],
                                         start=(k == 0), stop=(k == KC - 1))
                    nc.vector.tensor_copy(kT[0:64, cs], pk[0:64, :])

                # V projection -> vT, then PE-transpose into v tiles
                for ci in range(NC4):
                    cs = slice(ci * 512, (ci + 1) * 512)
                    pv_ = ppsum.tile([128, 512], F32, tag="proj")
                    for k in range(KC):
                        nc.tensor.matmul(pv_[:], wv[:, k, :], xT[:, k, cs],
                                         start=(k == 0), stop=(k == KC - 1))
                    nc.vector.tensor_copy(vT[0:64, cs], pv_[0:64, :])
                vtr = ppsum.tile([128, NT, 64], F16, tag="vtr")
                for t in range(NT):
                    nc.tensor.transpose(vtr[:, t, :],
                                        vT[:, t * 128:(t + 1) * 128], iden[:])
                    nc.vector.tensor_copy(v[:, t, 0:HD], vtr[:, t, :])

                # Q projection
                for hp in range(2):
                    for ci in range(NC4):
                        cs = slice(ci * 512, (ci + 1) * 512)
                        pq = ppsum.tile([128, 512], F32, tag="proj")
                        for k in range(KC):
                            nc.tensor.matmul(
                                pq[:], wq[:, k, hp * 128:(hp + 1) * 128],
                                xT[:, k, cs],
                                start=(k == 0), stop=(k == KC - 1))
                        nc.vector.tensor_copy(qT[hp][:, cs], pq[:])

                def rope_inplace(q_ap, nrows):
                    """q = q*cos + rot_half(q)*sin, on de-interleaved rows."""
                    rot = rope_pool.tile([128, T], F16, tag="rot", bufs=3)
                    for blk in range(nrows // 64):
                        r0 = blk * 64
                        nc.sync.dma_start(rot[r0:r0 + 32, :],
                                          q_ap[r0 + 32:r0 + 64, :])
                        nc.sync.dma_start(rot[r0 + 32:r0 + 64, :],
                                          q_ap[r0:r0 + 32, :])
                    nc.vector.tensor_mul(q_ap[0:nrows, :], q_ap[0:nrows, :],
                                         cosf[0:nrows, :])
                    nc.vector.tensor_mul(rot[0:nrows, :], rot[0:nrows, :],
                                         sinf[0:nrows, :])
                    nc.vector.tensor_add(q_ap[0:nrows, :], q_ap[0:nrows, :],
                                         rot[0:nrows, :])

                rope_inplace(kT[:], 64)
                nc.sync.dma_start(kT[64:128, :], kT[0:64, :])
                for hp in range(2):
                    rope_inplace(qT[hp][:], 128)

            # --- attention + out-projection, ci-outer / head-inner ---
            with (
                tc.tile_pool(name="aox", bufs=2) as aox,
                tc.tile_pool(name="at", bufs=3) as at_pool,
                tc.tile_pool(name="scp", bufs=2, space="PSUM") as scp,
                tc.tile_pool(name="pvp", bufs=2, space="PSUM") as pvp,
                tc.tile_pool(name="pop", bufs=1, space="PSUM") as pop,
                tc.tile_pool(name="outp", bufs=3) as outp,
            ):
                for ci in range(NC4):
                    cs = slice(ci * 512, (ci + 1) * 512)
                    n_tj = (ci + 1) * 4
                    for h in range(NQH):
                        hp, hr = divmod(h, 2)
                        qrow = slice(hr * 64, hr * 64 + 64)
                        pv = pvp.tile([128, 512], F32, tag="pv")
                        for tg in range(n_tj // G):
                            scg = scp.tile([128, G, 512], F32, tag="sc")
                            for j in range(G):
                                tj = tg * G + j
                                nc.tensor.matmul(
                                    scg[:, j, :],
                                    kT[qrow, tj * 128:(tj + 1) * 128],
                                    qT[hp][qrow, cs],
                                    start=True, stop=True)
                            atg = at_pool.tile([128, G, 512], F16, tag="at")
                            nc.scalar.activation(
                                atg[:], scg[:],
                                mybir.ActivationFunctionType.Exp,
                                scale=0.125, bias=bias[:])
                            for j in range(G):
                                tj = tg * G + j
                                if tj >= ci * 4:  # diagonal: causal 0/1 mask
                                    nc.vector.tensor_mul(
                                        atg[:, j, :], atg[:, j, :],
                                        msk[:, tj - ci * 4, :])
                            for j in range(G):
                                tj = tg * G + j
                                nc.tensor.matmul(
                                    pv[:], v[:, tj, :], atg[:, j, :],
                                    start=(tj == 0), stop=(tj == n_tj - 1))
                        # 1/L: pack L row -> [128,4], reciprocal, unpack
                        # (ACT copy: same table set as exp, no table switch;
                        # gpsimd cannot read PSUM)
                        lrow = aox.tile([1, 512], F32, tag="lrow")
                        nc.scalar.copy(lrow[:], pv[64:65, :])
                        pkl = aox.tile([128, 4], F32, tag="pkl")
                        nc.sync.dma_start(pkl[:], lrow[:])
                        rcl = aox.tile([128, 4], F16, tag="rcl")
                        with nc.allow_low_precision(reason="fp16 linv"):
                            nc.vector.reciprocal(rcl[:], pkl[:])
                        linv = aox.tile([1, 512], F16, tag="linv")
                        nc.sync.dma_start(linv[:], rcl[:])
                        # broadcast linv over 64 partitions via ones-matmul
                        lb = scp.tile([128, 512], F32, tag="sc")
                        nc.tensor.matmul(lb[:], onesr[:], linv[:],
                                         start=True, stop=True)
                        lbs = aox.tile([64, 512], F16, tag="lbs")
                        nc.vector.tensor_copy(lbs[:], lb[0:64, :])
                        if hr == 0:
                            dst = ao[hp][ci][0:64, :]
                        else:
                            dst = aox.tile([64, 512], F16, tag="aotmp")
                        nc.vector.tensor_mul(dst, pv[0:64, :], lbs[:])
                        if hr == 1:
                            nc.sync.dma_start(ao[hp][ci][64:128, :], dst)

                    # out-projection for this ci's four row tiles
                    for tt in range(4):
                        t = ci * 4 + tt
                        toff = slice(tt * 128, (tt + 1) * 128)
                        po = pop.tile([128, 2, 512], F32, tag="po")
                        for nh in range(2):
                            ns = slice(nh * 512, (nh + 1) * 512)
                            for cc in range(2):
                                nc.tensor.matmul(
                                    po[:, nh, :],
                                    ao[cc][ci][:, toff],
                                    wo[:, cc, ns],
                                    start=(cc == 0), stop=(cc == 1))
                        ot = outp.tile([128, D], F16, tag="ot")
                        nc.vector.tensor_copy(ot[:, 0:512], po[:, 0, :])
                        nc.scalar.copy(ot[:, 512:1024], po[:, 1, :])
                        nc.sync.dma_start(out_d[t * 128:(t + 1) * 128, :], ot[:])

    nc.compile()
    return nc


def make_in_maps(x, freqs_cos, freqs_sin, wq, wk, wv, wo):
    """Host-side sharding + layout prep. Returns per-core input dicts."""
    f16 = np.float16
    x = np.asarray(x, np.float32)
    fc = np.asarray(freqs_cos, np.float32)
    fs = np.asarray(freqs_sin, np.float32)
    wq = np.asarray(wq, np.float32)
    wk = np.asarray(wk, np.float32)
    wv = np.asarray(wv, np.float32)
    wo = np.asarray(wo, np.float32)

    perm = np.concatenate([np.arange(0, HD, 2), np.arange(1, HD, 2)])
    cosT = np.ascontiguousarray(fc.T)            # (32, T)
    sinT = np.ascontiguousarray(fs.T)
    cosf = np.concatenate([cosT] * 4, axis=0).astype(f16)    # (128, T)
    sinf = np.concatenate([-sinT, sinT, -sinT, sinT], axis=0).astype(f16)

    jj = np.arange(128)[:, None]
    ii = np.arange(512)[None, :]
    msk = np.stack(
        [np.where(r * 128 + jj <= ii, 1.0, 0.0) for r in range(4)], axis=0
    ).astype(f16)                                # (4, 128, 512)
    mskT = np.ascontiguousarray(msk.transpose(1, 0, 2))  # (128, 4, 512)
    iden = np.eye(64, dtype=f16)

    def pad128(w):  # (D, 64) -> (D, 128)
        z = np.zeros((D, 128), f16)
        z[:, 0:HD] = w
        return z

    in_maps = []
    for c in range(N_CORES):
        b, g = divmod(c, 4)
        wq_c = wq[:, g * QCOLS:(g + 1) * QCOLS]
        wq_c = np.ascontiguousarray(
            wq_c.reshape(D, NQH, HD)[:, :, perm].reshape(D, QCOLS)).astype(f16)
        wk_c = pad128(wk[:, g * HD:(g + 1) * HD][:, perm].astype(f16))
        wv_c = pad128(wv[:, g * HD:(g + 1) * HD].astype(f16))
        wo_c = np.ascontiguousarray(wo[g * QCOLS:(g + 1) * QCOLS, :]).astype(f16)
        xT_c = np.ascontiguousarray(x[b].T).astype(f16)
        in_maps.append({
            "xT": xT_c, "wq": wq_c, "wk": wk_c, "wv": wv_c, "wo": wo_c,
            "cosf": cosf, "sinf": sinf, "msk": mskT, "iden": iden,
        })
    return in_maps


def run_on_cores(in_maps, trace=False, **kwargs):
    if "nc" not in _cache:
        _cache["nc"] = build_nc()
    return run_bass_kernel_spmd(
        _cache["nc"], in_maps, core_ids=list(range(N_CORES)), trace=trace,
        **kwargs)


def kernel(x, freqs_cos, freqs_sin, wq, wk, wv, wo):
    in_maps = make_in_maps(x, freqs_cos, freqs_sin, wq, wk, wv, wo)
    res = run_on_cores(in_maps)
    outs = [np.asarray(res.results[c]["out"], np.float32)
            for c in range(N_CORES)]
    full = np.empty((B, T, D), np.float32)
    for b in range(B):
        full[b] = outs[4 * b] + outs[4 * b + 1] + outs[4 * b + 2] + outs[4 * b + 3]
    return full


# revision 9
# speedup vs baseline: 1.7651x; 1.5178x over previous
"""GQA attention kernel for Trainium2, 8 NeuronCores — fp16, software-pipelined.

Problem: B=2, T=2048, D=1024, 16 Q heads / 4 KV heads, head_dim=64, RoPE,
causal softmax, out-projection.

Sharding: 8 cores = 2 (batch) x 4 (KV group). Core c handles batch c//4 and
KV group g=c%4 (query heads 4g..4g+3). wq/wk/wv column-sharded, wo
row-sharded; the 4 partial outputs per batch are summed on the host.

All matmul operands are fp16 (PSUM accumulates fp32); rel err ~6e-4 vs the
fp64 reference. Every stationary operand is padded to 128 columns so Fast
Weight Load triggers and LDWEIGHTS hides under the previous matmul.

Layout is transposed (head_dim on partitions): xT (D,T), qT (256,T),
kT (64,T dup'd to 128), scoresT[j,i] = k_j.q_i. Softmax computes
exp(s/8 - 4): the bias keeps unnormalized weights in fp16 range and cancels
exactly through 1/L. Causality: the diagonal 512x512 region of each query
chunk is computed TRIMMED — the four 128-key blocks only cover query
columns [128r:512), and all four residual triangles are the SAME [128,128]
0/1 matrix, applied multiplicatively to the fp16 `at` tile after exp
(split over DVE and GpSimd; GpSimd cannot touch PSUM). L rides the PV
matmul as a ones-column of v; 1/L is computed lane-parallel by packing the
L row [1,512] into [128,4] via SBUF-SBUF DMA, then broadcast back over 64
partitions with a ones-stationary matmul.

Scheduling: engines are strict-FIFO, so emission order is the schedule.
The main loop runs query-chunk rounds (ci-outer, heads inner) and weaves
"filler" PE work — next round's K/V/Q projection chunks, previous round's
output-projection tiles — between attention groups, so the PE queue never
drains while ACT grinds exp (ACT is the attention-phase pacer at
(N+352)/1.2 ns). A drained PE triggers the HAM clock gate (PE drops
2.4 -> 1.2 GHz), which is what made previous versions 2x slow. Each head's
L-chain/normalize is deferred into the next head's stretch so its DMA
round-trips never block the ACT/DVE queues.
"""

import numpy as np
import sys
from collections import deque

sys.path.insert(0, "/opt/trn_rl_repo")

from concourse import bass, bacc, mybir, tile  # noqa: E402
from concourse.bass_utils import run_bass_kernel_spmd  # noqa: E402

F32 = mybir.dt.float32
F16 = mybir.dt.float16
EXP = mybir.ActivationFunctionType.Exp

B, T, D = 2, 2048, 1024
HD = 64                      # head dim
NQH = 4                      # query heads per core
QCOLS = NQH * HD             # 256
KC = D // 128                # 8 contraction chunks
NT = T // 128                # 16 row tiles
NC4 = T // 512               # 4 512-wide column chunks
N_CORES = 8

# trimmed diagonal geometry: block r covers query cols [TRIM_OFF[r]:512),
# packed into diag-a (r0,r1,r3) + diag-b (r2) exp groups.
TRIM_OFF = [0, 128, 256, 384]

_cache = {}


def build_nc():
    nc = bacc.Bacc("TRN2", target_bir_lowering=False, debug=False)

    xT_d = nc.declare_dram_parameter("xT", [D, T], F16, isOutput=False)
    wq_d = nc.declare_dram_parameter("wq", [D, QCOLS], F16, isOutput=False)
    wk_d = nc.declare_dram_parameter("wk", [D, 128], F16, isOutput=False)
    wv_d = nc.declare_dram_parameter("wv", [D, 128], F16, isOutput=False)
    wo_d = nc.declare_dram_parameter("wo", [QCOLS, D], F16, isOutput=False)
    cos_d = nc.declare_dram_parameter("cosf", [128, T], F16, isOutput=False)
    sin_d = nc.declare_dram_parameter("sinf", [128, T], F16, isOutput=False)
    tri_d = nc.declare_dram_parameter("tri", [128, 128], F16, isOutput=False)
    idn_d = nc.declare_dram_parameter("iden", [64, 64], F16, isOutput=False)
    out_d = nc.declare_dram_parameter("out", [T, D], F16, isOutput=True)

    with tile.TileContext(nc) as tc:
        with (
            tc.tile_pool(name="sb", bufs=1) as sb,
            tc.tile_pool(name="sbx", bufs=1) as sbx,
            tc.tile_pool(name="rope", bufs=2) as rope_pool,
            tc.tile_pool(name="at", bufs=3) as at_pool,
            tc.tile_pool(name="aox", bufs=2) as aox,
            tc.tile_pool(name="outp", bufs=3) as outp,
            tc.tile_pool(name="wrk", bufs=1, space="PSUM") as wrk,
            tc.tile_pool(name="scp", bufs=2, space="PSUM") as scp,
            tc.tile_pool(name="pvp", bufs=2, space="PSUM") as pvp,
        ):
            wq = sb.tile([128, KC, QCOLS], F16, tag="wq")
            wk = sb.tile([128, KC, 128], F16, tag="wk")
            wv = sb.tile([128, KC, 128], F16, tag="wv")
            wo = sb.tile([128, 2, D], F16, tag="wo")
            cosf = sb.tile([128, T], F16, tag="cosf")
            sinf = sb.tile([128, T], F16, tag="sinf")
            tri = sb.tile([128, 128], F16, tag="tri")
            iden = sb.tile([64, 64], F16, tag="iden")
            onesr = sb.tile([1, 128], F16, tag="onesr")
            bias = sb.tile([128, 1], F32, tag="bias")
            # per-512-chunk tiles (chunk-grain independence for the pipeline)
            qTc = [[sb.tile([128, 512], F16, tag=f"qT{hp}_{ci}",
                            name=f"qT{hp}_{ci}") for ci in range(NC4)]
                   for hp in range(2)]
            kTc = [sb.tile([128, 512], F16, tag=f"kT{ci}", name=f"kT{ci}")
                   for ci in range(NC4)]
            vTc = [sb.tile([64, 512], F16, tag=f"vT{ci}", name=f"vT{ci}")
                   for ci in range(NC4)]
            vc = [sb.tile([128, 4, 128], F16, tag=f"v{ci}", name=f"v{ci}")
                  for ci in range(NC4)]
            ao = [[sb.tile([128, 512], F16, tag=f"ao{hp}_{ci}",
                           name=f"ao{hp}_{ci}") for ci in range(NC4)]
                  for hp in range(2)]
            xT = sbx.tile([128, KC, T], F16, tag="xT")

            nc.gpsimd.memset(onesr[:], 1.0)
            nc.gpsimd.memset(bias[:], -4.0)
            for ci in range(NC4):
                nc.gpsimd.memset(vc[ci][:], 0.0)
                nc.gpsimd.memset(vc[ci][:, :, HD:HD + 1], 1.0)

            for k in range(KC):
                nc.sync.dma_start(xT[:, k, :], xT_d[k * 128:(k + 1) * 128, :])
            for k in range(KC):
                nc.sync.dma_start(wk[:, k, :], wk_d[k * 128:(k + 1) * 128, :])
                nc.sync.dma_start(wv[:, k, :], wv_d[k * 128:(k + 1) * 128, :])
                nc.sync.dma_start(wq[:, k, :], wq_d[k * 128:(k + 1) * 128, :])
            nc.sync.dma_start(cosf[:], cos_d[:])
            nc.sync.dma_start(sinf[:], sin_d[:])
            nc.sync.dma_start(tri[:], tri_d[:])
            nc.sync.dma_start(iden[:], idn_d[:])
            for c in range(2):
                nc.sync.dma_start(wo[:, c, :], wo_d[c * 128:(c + 1) * 128, :])

            # ---------- emission helpers ----------
            def rope_chunk(q_ap, nrows, cs):
                """in-place RoPE on a [*, 512] chunk tile (cs indexes cos/sin)."""
                rot = rope_pool.tile([128, 512], F16, tag="rot", bufs=4)
                for blk in range(nrows // 64):
                    r0 = blk * 64
                    nc.gpsimd.dma_start(rot[r0:r0 + 32, :],
                                        q_ap[r0 + 32:r0 + 64, :])
                    nc.gpsimd.dma_start(rot[r0 + 32:r0 + 64, :],
                                        q_ap[r0:r0 + 32, :])
                nc.vector.tensor_mul(q_ap[0:nrows, :], q_ap[0:nrows, :],
                                     cosf[0:nrows, cs])
                nc.vector.tensor_mul(rot[0:nrows, :], rot[0:nrows, :],
                                     sinf[0:nrows, cs])
                nc.vector.tensor_add(q_ap[0:nrows, :], q_ap[0:nrows, :],
                                     rot[0:nrows, :])

            def proj_k(ci):
                cs = slice(ci * 512, (ci + 1) * 512)
                p = wrk.tile([128, 2, 512], F32, tag="wk_")
                for k in range(KC):
                    nc.tensor.matmul(p[:, 0, :], wk[:, k, :], xT[:, k, cs],
                                     start=(k == 0), stop=(k == KC - 1))
                nc.vector.tensor_copy(kTc[ci][0:64, :], p[0:64, 0, :])
                rope_chunk(kTc[ci], 64, cs)
                nc.sync.dma_start(kTc[ci][64:128, :], kTc[ci][0:64, :])

            def proj_v(ci):
                cs = slice(ci * 512, (ci + 1) * 512)
                p = wrk.tile([128, 2, 512], F32, tag="wk_")
                for k in range(KC):
                    nc.tensor.matmul(p[:, 0, :], wv[:, k, :], xT[:, k, cs],
                                     start=(k == 0), stop=(k == KC - 1))
                nc.vector.tensor_copy(vTc[ci][0:64, :], p[0:64, 0, :])

            def trans_v(ci):
                vtr = wrk.tile([128, 4, 64], F16, tag="wk_")
                for tt in range(4):
                    nc.tensor.transpose(vtr[:, tt, :],
                                        vTc[ci][:, tt * 128:(tt + 1) * 128],
                                        iden[:])
                    nc.vector.tensor_copy(vc[ci][:, tt, 0:HD], vtr[:, tt, :])

            def proj_q(hp, ci):
                cs = slice(ci * 512, (ci + 1) * 512)
                p = wrk.tile([128, 2, 512], F32, tag="wk_")
                for k in range(KC):
                    nc.tensor.matmul(
                        p[:, 0, :], wq[:, k, hp * 128:(hp + 1) * 128],
                        xT[:, k, cs], start=(k == 0), stop=(k == KC - 1))
                nc.vector.tensor_copy(qTc[hp][ci][:], p[:, 0, :])
                rope_chunk(qTc[hp][ci], 128, cs)

            def po_tile(t):
                ci, tt = divmod(t, 4)
                toff = slice(tt * 128, (tt + 1) * 128)
                po = wrk.tile([128, 2, 512], F32, tag="wk_")
                for nh in range(2):
                    ns = slice(nh * 512, (nh + 1) * 512)
                    for cc in range(2):
                        nc.tensor.matmul(
                            po[:, nh, :], ao[cc][ci][:, toff], wo[:, cc, ns],
                            start=(cc == 0), stop=(cc == 1))
                ot = outp.tile([128, D], F16, tag="ot")
                nc.vector.tensor_copy(ot[:, 0:512], po[:, 0, :])
                nc.vector.tensor_copy(ot[:, 512:1024], po[:, 1, :])
                nc.sync.dma_start(out_d[t * 128:(t + 1) * 128, :], ot[:])

            def kchunk(tj):
                """[64 or 128, 128] stationary slice for key tile tj."""
                return kTc[tj // 4], (tj % 4) * 128

            def attention_head(ci, h):
                """Scores+exp+mask+PV for head h, query chunk ci.
                Returns a closure finishing the 1/L + normalize (deferred)."""
                hp, hr = divmod(h, 2)
                qrow = slice(hr * 64, hr * 64 + 64)
                qt = qTc[hp][ci]
                pv = pvp.tile([128, 512], F32, tag="pv")
                n_off = ci * 4  # number of off-diagonal key tiles
                # off-diagonal pairs
                for tg in range(n_off // 2):
                    scg = scp.tile([128, 2, 512], F32, tag="sc")
                    for j in range(2):
                        tj = tg * 2 + j
                        kt, ko = kchunk(tj)
                        nc.tensor.matmul(
                            scg[:, j, :], kt[qrow, ko:ko + 128], qt[qrow, :],
                            start=True, stop=True)
                    atg = at_pool.tile([128, 2, 512], F16, tag="at")
                    nc.scalar.activation(atg[:], scg[:], EXP,
                                         scale=0.125, bias=bias[:])
                    for j in range(2):
                        tj = tg * 2 + j
                        nc.tensor.matmul(pv[:], vc[tj // 4][:, tj % 4, :],
                                         atg[:, j, :],
                                         start=(tj == 0), stop=False)
                # diagonal, trimmed: r0@[0:512], r1@[512:896], r3@[896:1024]
                kt, _ = kchunk(n_off)  # all diag tiles live in kTc[ci]
                sca = scp.tile([128, 2, 512], F32, tag="sc")
                nc.tensor.matmul(sca[:, 0, :], kt[qrow, 0:128],
                                 qt[qrow, :], start=True, stop=True)
                nc.tensor.matmul(sca[:, 1, 0:384], kt[qrow, 128:256],
                                 qt[qrow, 128:512], start=True, stop=True)
                nc.tensor.matmul(sca[:, 1, 384:512], kt[qrow, 384:512],
                                 qt[qrow, 384:512], start=True, stop=True)
                ata = at_pool.tile([128, 2, 512], F16, tag="at")
                nc.scalar.activation(ata[:], sca[:], EXP,
                                     scale=0.125, bias=bias[:])
                # diag-b: r2 (256 wide)
                scb = scp.tile([128, 2, 512], F32, tag="sc")
                nc.tensor.matmul(scb[:, 0, 0:256], kt[qrow, 256:384],
                                 qt[qrow, 256:512], start=True, stop=True)
                atb = at_pool.tile([128, 256], F16, tag="at")
                nc.scalar.activation(atb[:], scb[:, 0, 0:256], EXP,
                                     scale=0.125, bias=bias[:])
                # residual triangles (identical [128,128] 0/1 matrix)
                nc.vector.tensor_mul(ata[:, 0, 0:128], ata[:, 0, 0:128], tri[:])
                nc.gpsimd.tensor_mul(ata[:, 1, 0:128], ata[:, 1, 0:128], tri[:])
                nc.gpsimd.tensor_mul(ata[:, 1, 384:512], ata[:, 1, 384:512],
                                     tri[:])
                nc.vector.tensor_mul(atb[:, 0:128], atb[:, 0:128], tri[:])
                # PV over the diagonal blocks (r3 before r2; stop on r2)
                vd = vc[ci]
                nc.tensor.matmul(pv[:, 0:512], vd[:, 0, :], ata[:, 0, :],
                                 start=(n_off == 0), stop=False)
                nc.tensor.matmul(pv[:, 128:512], vd[:, 1, :], ata[:, 1, 0:384],
                                 start=False, stop=False)
                nc.tensor.matmul(pv[:, 384:512], vd[:, 3, :],
                                 ata[:, 1, 384:512], start=False, stop=False)
                nc.tensor.matmul(pv[:, 256:512], vd[:, 2, :], atb[:],
                                 start=False, stop=True)

                def lchain():
                    lrow = aox.tile([1, 512], F32, tag="lrow")
                    nc.scalar.copy(lrow[:], pv[64:65, :])
                    pkl = aox.tile([128, 4], F32, tag="pkl")
                    nc.gpsimd.dma_start(pkl[:], lrow[:])
                    rcl = aox.tile([128, 4], F16, tag="rcl")
                    with nc.allow_low_precision(reason="fp16 linv"):
                        nc.vector.reciprocal(rcl[:], pkl[:])
                    linv = aox.tile([1, 512], F16, tag="linv")
                    nc.gpsimd.dma_start(linv[:], rcl[:])
                    lb = scp.tile([128, 512], F32, tag="sc")
                    nc.tensor.matmul(lb[:], onesr[:], linv[:],
                                     start=True, stop=True)
                    lbs = aox.tile([64, 512], F16, tag="lbs")
                    nc.vector.tensor_copy(lbs[:], lb[0:64, :])
                    if hr == 0:
                        dst = ao[hp][ci][0:64, :]
                    else:
                        dst = aox.tile([64, 512], F16, tag="aotmp")
                    nc.vector.tensor_mul(dst, pv[0:64, :], lbs[:])
                    if hr == 1:
                        nc.sync.dma_start(ao[hp][ci][64:128, :], dst)
                return lchain

            # ---------- schedule ----------
            # prologue: chunk-0 projections
            proj_k(0)
            proj_v(0)
            trans_v(0)
            proj_q(0, 0)
            proj_q(1, 0)

            # projf: (due_chunk, closure) — MUST be emitted before round
            # `due_chunk` starts (program order defines the data hazards).
            # pof: out-projection fillers, no deadline until the tail.
            projf = deque()
            pof = deque()

            def pop_filler():
                if projf:
                    projf.popleft()[1]()
                elif pof:
                    pof.popleft()()

            pending = None
            for ci in range(NC4):
                while projf and projf[0][0] <= ci:   # safety drain
                    projf.popleft()[1]()
                if ci + 1 < NC4:
                    c = ci + 1
                    projf.append((c, lambda c=c: proj_k(c)))
                    projf.append((c, lambda c=c: proj_v(c)))
                    projf.append((c, lambda c=c: trans_v(c)))
                    projf.append((c, lambda c=c: proj_q(0, c)))
                    projf.append((c, lambda c=c: proj_q(1, c)))
                for h in range(NQH):
                    nxt = attention_head(ci, h)
                    if pending is not None:
                        pending()
                    pending = nxt
                    pop_filler()
                    pop_filler()
                # close the round: last head's L-chain, then queue out-proj
                pending()
                pending = None
                for tt in range(4):
                    pof.append(lambda t=ci * 4 + tt: po_tile(t))
            while projf or pof:
                pop_filler()

    nc.compile()
    return nc


def make_in_maps(x, freqs_cos, freqs_sin, wq, wk, wv, wo):
    """Host-side sharding + layout prep. Returns per-core input dicts."""
    f16 = np.float16
    x = np.asarray(x, np.float32)
    fc = np.asarray(freqs_cos, np.float32)
    fs = np.asarray(freqs_sin, np.float32)
    wq = np.asarray(wq, np.float32)
    wk = np.asarray(wk, np.float32)
    wv = np.asarray(wv, np.float32)
    wo = np.asarray(wo, np.float32)

    perm = np.concatenate([np.arange(0, HD, 2), np.arange(1, HD, 2)])
    cosT = np.ascontiguousarray(fc.T)            # (32, T)
    sinT = np.ascontiguousarray(fs.T)
    cosf = np.concatenate([cosT] * 4, axis=0).astype(f16)    # (128, T)
    sinf = np.concatenate([-sinT, sinT, -sinT, sinT], axis=0).astype(f16)

    jj = np.arange(128)[:, None]
    cc_ = np.arange(128)[None, :]
    tri = (jj <= cc_).astype(f16)                # [key j, query c]
    iden = np.eye(64, dtype=f16)

    def pad128(w):  # (D, 64) -> (D, 128)
        z = np.zeros((D, 128), f16)
        z[:, 0:HD] = w
        return z

    in_maps = []
    for c in range(N_CORES):
        b, g = divmod(c, 4)
        wq_c = wq[:, g * QCOLS:(g + 1) * QCOLS]
        wq_c = np.ascontiguousarray(
            wq_c.reshape(D, NQH, HD)[:, :, perm].reshape(D, QCOLS)).astype(f16)
        wk_c = pad128(wk[:, g * HD:(g + 1) * HD][:, perm].astype(f16))
        wv_c = pad128(wv[:, g * HD:(g + 1) * HD].astype(f16))
        wo_c = np.ascontiguousarray(wo[g * QCOLS:(g + 1) * QCOLS, :]).astype(f16)
        xT_c = np.ascontiguousarray(x[b].T).astype(f16)
        in_maps.append({
            "xT": xT_c, "wq": wq_c, "wk": wk_c, "wv": wv_c, "wo": wo_c,
            "cosf": cosf, "sinf": sinf, "tri": tri, "iden": iden,
        })
    return in_maps


def run_on_cores(in_maps, trace=False, **kwargs):
    if "nc" not in _cache:
        _cache["nc"] = build_nc()
    return run_bass_kernel_spmd(
        _cache["nc"], in_maps, core_ids=list(range(N_CORES)), trace=trace,
        **kwargs)


def kernel(x, freqs_cos, freqs_sin, wq, wk, wv, wo):
    in_maps = make_in_maps(x, freqs_cos, freqs_sin, wq, wk, wv, wo)
    res = run_on_cores(in_maps)
    outs = [np.asarray(res.results[c]["out"], np.float32)
            for c in range(N_CORES)]
    full = np.empty((B, T, D), np.float32)
    for b in range(B):
        full[b] = outs[4 * b] + outs[4 * b + 1] + outs[4 * b + 2] + outs[4 * b + 3]
    return full
